# revision 1
# baseline (speedup 1.0000x reference)
"""CrossBidirectionalAttention Trainium2 kernel.

Problem (reference.py):
  B=2, L=S=2048, C=1024, H=16, HD=64
  qk0 = split_heads(x0 @ Wqk.T) * temp^0.5 ; qk1 likewise
  sim = einsum('blhd,bshd->bhls', qk0, qk1)
  o0 = softmax(sim, -1) @ v1 ; o1 = softmax(sim, -2)^T-contract @ v0
  return (merge(o0) @ Wmerge.T, merge(o1) @ Wmerge.T)

Sharding: 8 cores = 2 batches x 4 head-groups (4 heads each). Each core
computes its (b, head-group) slice end-to-end, producing partial merged
outputs (sum over its heads' columns of Wmerge); host sums the 4 partials
per batch. No max-subtraction is needed in softmax: sim ~ N(0,1), so
exp(temp*sim) <= ~e^6, safely in fp32/bf16 range. Normalization sums are
obtained for free as a 65th ones-column in the attention matmuls' lhsT.

Device-side dataflow per core (all matmul inputs bf16, PSUM fp32):
  x0T, x1T [128,8,2048]  (c_in on partitions; host pre-transposed)
  qk projections -> qk0,qk1 [128,2,2048] (head-cols on partitions)
  v projections  -> v0,v1 [128,16,4,65]  (seq on partitions; col 64 = ones)
  per head h: sim l-tiles (K=64 matmul) -> exp (ACT, scale=temp) -> E bf16
              o1T accumulation: lhsT=[v0_h|1] [128,65], rhs=E   (K=l)
              E^T via SBUF->SBUF DMA xbar transpose
              o0T accumulation: lhsT=[v1_h|1] [128,65], rhs=E^T (K=s)
              row 64 of each accumulator = softmax denominators;
              normalize via reciprocal + gpsimd partition_broadcast
  merge: lhsT=o{0,1}nT [128,2,2048], rhs=Wmerge slice -> out [l,c] fp32
"""

import os
import numpy as np
import ml_dtypes

B, L, S, C, H = 2, 2048, 2048, 1024, 16
HD = C // H  # 64
TEMP = float(HD) ** -0.5
N_CORES = 8
BF16 = ml_dtypes.bfloat16

_NC_CACHE = {}


def _build(lt_count=16, st_count=16):
    """Build the (identical-on-every-core) Bass program.

    lt_count/st_count: number of 128-row tiles of L and S (16 for the real
    problem; smaller for CoreSim validation).
    """
    import concourse.tile as tile
    from concourse import bacc, mybir

    Lc = lt_count * 128
    Sc = st_count * 128

    def chunks(total):
        # psum-tile-sized chunks (<=1024) each split into <=512 matmul subchunks
        out = []
        pos = 0
        while pos < total:
            clen = min(1024, total - pos)
            subs = []
            sp = 0
            while sp < clen:
                slen = min(512, clen - sp)
                subs.append((sp, slen))
                sp += slen
            out.append((pos, clen, subs))
            pos += clen
        return out

    f32 = mybir.dt.float32
    f32r = mybir.dt.float32r
    bf16 = mybir.dt.bfloat16

    nc = bacc.Bacc(None, target_bir_lowering=False, enable_partition_id=False)

    x0t_d = nc.dram_tensor("x0t", [128, 8, Lc], bf16, kind="ExternalInput")
    x1t_d = nc.dram_tensor("x1t", [128, 8, Sc], bf16, kind="ExternalInput")
    wqk_d = nc.dram_tensor("wqk", [128, 8, 256], bf16, kind="ExternalInput")
    wv_d = nc.dram_tensor("wv", [128, 8, 256], bf16, kind="ExternalInput")
    wm_d = nc.dram_tensor("wm", [128, 2, 1024], bf16, kind="ExternalInput")
    out0_d = nc.dram_tensor("out0", [128, lt_count, 1024], bf16, kind="ExternalOutput")
    out1_d = nc.dram_tensor("out1", [128, st_count, 1024], bf16, kind="ExternalOutput")

    with tile.TileContext(nc) as tc:
        with (
            tc.tile_pool(name="w", bufs=1) as wpool,
            tc.tile_pool(name="qk", bufs=1) as qkpool,
            tc.tile_pool(name="v", bufs=1) as vpool,
            tc.tile_pool(name="ont", bufs=1) as opool,
            tc.tile_pool(name="E", bufs=5) as epool,
            tc.tile_pool(name="et", bufs=1) as etpool,
            tc.tile_pool(name="small", bufs=2) as smallpool,
            tc.tile_pool(name="ostage", bufs=4) as ostagepool,
            tc.tile_pool(name="ps", bufs=4, space="PSUM") as pspool,
        ):
            wqk_t = wpool.tile([128, 8, 256], bf16)
            wv_t = wpool.tile([128, 8, 256], bf16)
            wm_t = wpool.tile([128, 2, 1024], bf16)
            nc.sync.dma_start(wqk_t[:], wqk_d[:])
            nc.sync.dma_start(wv_t[:], wv_d[:])

            qk0 = qkpool.tile([128, 4, Lc], bf16)  # per-head, dup on halves
            qk1 = qkpool.tile([128, 4, Sc], bf16)
            v0 = vpool.tile([128, lt_count, 4, 65], bf16)
            v1 = vpool.tile([128, st_count, 4, 65], bf16)
            o0nT = opool.tile([128, 2, Lc], bf16)
            o1nT = opool.tile([128, 2, Sc], bf16)

            nc.vector.memset(v0[:, :, :, 64:65], 1.0)
            nc.vector.memset(v1[:, :, :, 64:65], 1.0)
            ones_f32 = wpool.tile([1, 64], f32)
            nc.vector.memset(ones_f32[:], 1.0)
            ones1x64 = wpool.tile([1, 64], f32r)
            nc.vector.tensor_copy(ones1x64[:], ones_f32[:])

            # ---------------- projections ----------------
            # One x tensor resident at a time (they share the "big" tag slot).
            # both x tensors share the (later) ET slot; loads run in parallel
            xboth = etpool.tile([128, 16, max(Lc, Sc)], bf16, tag="et")
            for kc in range(0, 8, 2):
                nc.sync.dma_start(
                    xboth[:, kc : kc + 2, 0:Sc], x1t_d[:, kc : kc + 2, :]
                )
            for kc in range(0, 8, 2):
                nc.sync.dma_start(
                    xboth[:, 8 + kc : 8 + kc + 2, 0:Lc], x0t_d[:, kc : kc + 2, :]
                )
            nc.sync.dma_start(wm_t[:], wm_d[:])
            x1T = xboth[:, 0:8, 0:Sc]
            x0T = xboth[:, 8:16, 0:Lc]
            for xt, n_seq, qk_t, v_t in (
                (x1T, st_count, qk1, v1),
                (x0T, lt_count, qk0, v0),
            ):
                # qkT: [head-col, seq] ; m-tiles of 128 head cols
                for m in range(2):
                    for cpos, clen, subs in chunks(n_seq * 128):
                        ps = pspool.tile([128, 1024], f32, tag="ps")
                        for sp, slen in subs:
                            for k in range(8):
                                nc.tensor.matmul(
                                    ps[:, sp : sp + slen],
                                    wqk_t[:, k, m * 128 : (m + 1) * 128],
                                    xt[:, k, cpos + sp : cpos + sp + slen],
                                    start=(k == 0),
                                    stop=(k == 7),
                                )
                        # duplicate each head's rows onto both partition halves
                        nc.vector.tensor_copy(
                            qk_t[0:64, 2 * m, cpos : cpos + clen], ps[0:64, 0:clen]
                        )
                        nc.scalar.copy(
                            qk_t[64:128, 2 * m, cpos : cpos + clen], ps[0:64, 0:clen]
                        )
                        nc.scalar.copy(
                            qk_t[0:64, 2 * m + 1, cpos : cpos + clen], ps[64:128, 0:clen]
                        )
                        nc.vector.tensor_copy(
                            qk_t[64:128, 2 * m + 1, cpos : cpos + clen], ps[64:128, 0:clen]
                        )
                # v: [seq, head*65] ; mt-tiles of 128 seq rows
                for mt in range(n_seq):
                    ps = pspool.tile([128, 1024], f32, tag="ps")
                    for k in range(8):
                        nc.tensor.matmul(
                            ps[:, 0:256],
                            xt[:, k, mt * 128 : (mt + 1) * 128],
                            wv_t[:, k, :],
                            start=(k == 0),
                            stop=(k == 7),
                        )
                    nc.vector.tensor_copy(
                        v_t[:, mt, :, 0:64],
                        ps[:, 0:256].rearrange("p (h d) -> p h d", h=4),
                    )

            # ---------------- per-head attention ----------------
            # E^T: ET[sp, st, l] = E[l, st*128+sp] (reuses x0T's slot)
            ET = etpool.tile([128, st_count, Lc], bf16, tag="et")
            for h in range(4):
                hp = (h % 2) * 64
                hm = h // 2
                s_chunks = chunks(Sc)
                l_chunks = chunks(Lc)
                po1 = [
                    pspool.tile([128, 1024], f32, tag="ps", name=f"po1_{h}_{i}")
                    for i in range(len(s_chunks))
                ]

                for t0 in range(0, lt_count, 2):
                    # pair two l-tiles on the two PE row-group halves (K=64 each)
                    pair = [(t0, 0)] + ([(t0 + 1, 64)] if t0 + 1 < lt_count else [])
                    e_ts = {
                        lt: epool.tile([128, Sc], bf16, tag="E", name=f"e_{h}_{lt}")
                        for lt, _ in pair
                    }
                    for cpos, clen, subs in s_chunks:
                        pss = {
                            lt: pspool.tile(
                                [128, 1024], f32, tag="ps", name=f"sim_{h}_{lt}_{cpos}"
                            )
                            for lt, _ in pair
                        }
                        for sp, slen in subs:
                            for lt, hp2 in pair:
                                nc.tensor.matmul(
                                    pss[lt][:, sp : sp + slen],
                                    qk0[hp2 : hp2 + 64, h, lt * 128 : (lt + 1) * 128],
                                    qk1[hp2 : hp2 + 64, h, cpos + sp : cpos + sp + slen],
                                    start=True,
                                    stop=True,
                                    tile_position=(hp2, 0),
                                )
                        for lt, _ in pair:
                            nc.scalar.activation(
                                e_ts[lt][:, cpos : cpos + clen],
                                pss[lt][:, 0:clen],
                                mybir.ActivationFunctionType.Exp,
                                scale=TEMP,
                            )
                    for lt, _ in pair:
                        e_t = e_ts[lt]
                        # o1 accumulation step for this lt
                        for oc, (cpos, clen, subs) in enumerate(s_chunks):
                            for sp, slen in subs:
                                nc.tensor.matmul(
                                    po1[oc][0:65, sp : sp + slen],
                                    v0[:, lt, h, :],
                                    e_t[:, cpos + sp : cpos + sp + slen],
                                    start=(lt == 0),
                                    stop=(lt == lt_count - 1),
                                )
                        # E^T tile
                        nc.sync.dma_start_transpose(
                            ET[:, :, lt * 128 : (lt + 1) * 128], e_t[:]
                        )

                # normalize o1T -> o1nT (row 64 = colsum)
                for oc, (cpos, clen, subs) in enumerate(s_chunks):
                    rc = smallpool.tile([1, 1024], f32r, tag="rc")
                    rcb = smallpool.tile([64, 1024], f32, tag="rcb")
                    bps = pspool.tile([128, 1024], f32, tag="ps", name=f"bps1_{h}_{oc}")
                    with nc.allow_low_precision(reason="f32r reciprocal for PE broadcast"):
                        nc.vector.reciprocal(rc[:, 0:clen], po1[oc][64:65, 0:clen])
                    for sp, slen in subs:
                        nc.tensor.matmul(
                            bps[0:64, sp : sp + slen], ones1x64[:],
                            rc[:, sp : sp + slen],
                            start=True, stop=True,
                        )
                    nc.vector.tensor_copy(rcb[:, 0:clen], bps[0:64, 0:clen])
                    nc.vector.tensor_tensor(
                        o1nT[hp : hp + 64, hm, cpos : cpos + clen],
                        po1[oc][0:64, 0:clen],
                        rcb[:, 0:clen],
                        mybir.AluOpType.mult,
                    )

                # o0 accumulation over st
                for oc, (cpos, clen, subs) in enumerate(l_chunks):
                    po0 = pspool.tile([128, 1024], f32, tag="ps", name=f"po0_{h}_{oc}")
                    for st in range(st_count):
                        for sp, slen in subs:
                            nc.tensor.matmul(
                                po0[0:65, sp : sp + slen],
                                v1[:, st, h, :],
                                ET[:, st, cpos + sp : cpos + sp + slen],
                                start=(st == 0),
                                stop=(st == st_count - 1),
                            )
                    # normalize o0T -> o0nT (row 64 = rowsum)
                    rc = smallpool.tile([1, 1024], f32r, tag="rc")
                    rcb = smallpool.tile([64, 1024], f32, tag="rcb")
                    bps = pspool.tile([128, 1024], f32, tag="ps", name=f"bps0_{h}_{oc}")
                    with nc.allow_low_precision(reason="f32r reciprocal for PE broadcast"):
                        nc.vector.reciprocal(rc[:, 0:clen], po0[64:65, 0:clen])
                    for sp, slen in subs:
                        nc.tensor.matmul(
                            bps[0:64, sp : sp + slen], ones1x64[:],
                            rc[:, sp : sp + slen],
                            start=True, stop=True,
                        )
                    if h == 3:
                        nc.scalar.copy(rcb[:, 0:clen], bps[0:64, 0:clen])
                    else:
                        nc.vector.tensor_copy(rcb[:, 0:clen], bps[0:64, 0:clen])
                    nc.vector.tensor_tensor(
                        o0nT[hp : hp + 64, hm, cpos : cpos + clen],
                        po0[0:64, 0:clen],
                        rcb[:, 0:clen],
                        mybir.AluOpType.mult,
                    )

            # ---------------- merge ----------------
            for src, dst, n_seq in ((o1nT, out1_d, st_count), (o0nT, out0_d, lt_count)):
                for mt0 in range(0, n_seq, 2):
                    mts = [mt for mt in (mt0, mt0 + 1) if mt < n_seq]
                    st_t = ostagepool.tile([128, 2, 1024], bf16, tag="ostage")
                    for j, mt in enumerate(mts):
                        ps = pspool.tile([128, 1024], f32, tag="ps")
                        for c2 in range(2):
                            for k in range(2):
                                nc.tensor.matmul(
                                    ps[:, c2 * 512 : (c2 + 1) * 512],
                                    src[:, k, mt * 128 : (mt + 1) * 128],
                                    wm_t[:, k, c2 * 512 : (c2 + 1) * 512],
                                    start=(k == 0),
                                    stop=(k == 1),
                                )
                        if j == 0:
                            nc.vector.tensor_copy(st_t[:, j, :], ps[:])
                        else:
                            nc.scalar.copy(st_t[:, j, :], ps[:])
                    nc.sync.dma_start(
                        dst[:, mt0 : mt0 + len(mts), :], st_t[:, 0 : len(mts), :]
                    )

    nc.compile()
    return nc


def _get_nc(lt_count=16, st_count=16):
    key = (lt_count, st_count)
    if key not in _NC_CACHE:
        _NC_CACHE[key] = _build(lt_count, st_count)
    return _NC_CACHE[key]


def _shard_inputs(x0, x1, Wqk, Wv, Wmerge, lt_count=16, st_count=16):
    """Host-side prep: per-core transposed bf16 shards."""
    Lc, Sc = lt_count * 128, st_count * 128
    in_maps = []
    for c in range(N_CORES):
        b = c // 4
        hg = c % 4
        hs = slice(hg * 256, (hg + 1) * 256)
        x0t = x0[b].T.reshape(8, 128, Lc).transpose(1, 0, 2)
        x1t = x1[b].T.reshape(8, 128, Sc).transpose(1, 0, 2)
        wqk = Wqk[hs, :].T.reshape(8, 128, 256).transpose(1, 0, 2)
        wv = Wv[hs, :].T.reshape(8, 128, 256).transpose(1, 0, 2)
        wm = Wmerge[:, hs].T.reshape(2, 128, 1024).transpose(1, 0, 2)
        in_maps.append(
            {
                "x0t": np.ascontiguousarray(x0t).astype(BF16),
                "x1t": np.ascontiguousarray(x1t).astype(BF16),
                "wqk": np.ascontiguousarray(wqk).astype(BF16),
                "wv": np.ascontiguousarray(wv).astype(BF16),
                "wm": np.ascontiguousarray(wm).astype(BF16),
            }
        )
    return in_maps


def _gather_outputs(results, lt_count=16, st_count=16):
    Lc, Sc = lt_count * 128, st_count * 128
    o0 = np.zeros((B, Lc, C), np.float32)
    o1 = np.zeros((B, Sc, C), np.float32)
    for c, res in enumerate(results):
        b = c // 4
        o0[b] += res["out0"].astype(np.float32).transpose(1, 0, 2).reshape(Lc, C)
        o1[b] += res["out1"].astype(np.float32).transpose(1, 0, 2).reshape(Sc, C)
    return o0, o1


def kernel(x0, x1, Wqk, Wv, Wmerge):
    from concourse.bass_utils import run_bass_kernel_spmd

    x0 = np.asarray(x0, dtype=np.float32)
    x1 = np.asarray(x1, dtype=np.float32)
    Wqk = np.asarray(Wqk, dtype=np.float32)
    Wv = np.asarray(Wv, dtype=np.float32)
    Wmerge = np.asarray(Wmerge, dtype=np.float32)

    nc = _get_nc()
    in_maps = _shard_inputs(x0, x1, Wqk, Wv, Wmerge)
    trace = os.environ.get("BENCH_TRACE", "") == "1"
    res = run_bass_kernel_spmd(
        nc, in_maps, core_ids=list(range(N_CORES)), trace=trace
    )
    if trace and res.exec_time_ns is not None:
        print(f"HW exec time: {res.exec_time_ns} ns")
        if res.instructions_and_trace is not None:
            print(f"trace: {res.instructions_and_trace[1]}")
    return _gather_outputs(res.results)


# ---------------------------------------------------------------------------
# Timing harness (test.py only): repeated steady-state executions of the
# jitted SPMD body with device-resident inputs, calibrated against a trivial
# kernel measured the same way to subtract axon dispatch/RPC overhead.

def _make_runner(nc, in_maps):
    import jax
    import numpy as np
    from jax.sharding import Mesh, PartitionSpec
    from jax.experimental.shard_map import shard_map
    from concourse import bass2jax, mybir

    bass2jax.install_neuronx_cc_hook()

    in_names, out_names, out_avals, zero_outs = [], [], [], []
    for alloc in nc.m.functions[0].allocations:
        if not isinstance(alloc, mybir.MemoryLocationSet):
            continue
        name = alloc.memorylocations[0].name
        if alloc.kind == "ExternalInput":
            in_names.append(name)
        elif alloc.kind == "ExternalOutput":
            out_names.append(name)
            dt = mybir.dt.np(alloc.dtype)
            out_avals.append(
                jax.core.ShapedArray(tuple(alloc.tensor_shape), dt)
            )
            zero_outs.append(np.zeros(tuple(alloc.tensor_shape), dt))
    n_params = len(in_names)
    all_names = in_names + out_names

    def _body(*args):
        outs = bass2jax._bass_exec_p.bind(
            *args,
            out_avals=tuple(out_avals),
            in_names=tuple(all_names),
            out_names=tuple(out_names),
            lowering_input_output_aliases=(),
            sim_require_finite=True,
            sim_require_nnan=True,
            nc=nc,
        )
        return tuple(outs)

    n_cores = len(in_maps)
    devices = jax.devices()[:n_cores]
    mesh = Mesh(np.asarray(devices), ("core",))
    n_out = len(out_names)
    sharded = jax.jit(
        shard_map(
            _body,
            mesh=mesh,
            in_specs=(PartitionSpec("core"),) * (n_params + n_out),
            out_specs=(PartitionSpec("core"),) * n_out,
            check_rep=False,
        ),
        keep_unused=True,
    )
    concat_in = [
        np.concatenate([np.asarray(in_maps[c][nm]) for c in range(n_cores)], axis=0)
        for nm in in_names
    ]
    concat_zero = [
        np.zeros((n_cores * z.shape[0], *z.shape[1:]), z.dtype) for z in zero_outs
    ]
    dev_args = [jax.device_put(a) for a in concat_in + concat_zero]

    def run():
        outs = sharded(*dev_args)
        jax.block_until_ready(outs)
        return outs

    return run


def _trivial_nc():
    import concourse.tile as tile
    from concourse import bacc, mybir

    nc = bacc.Bacc(None, target_bir_lowering=False, enable_partition_id=False)
    a_d = nc.dram_tensor("tin", [128, 128], mybir.dt.float32, kind="ExternalInput")
    o_d = nc.dram_tensor("tout", [128, 128], mybir.dt.float32, kind="ExternalOutput")
    with tile.TileContext(nc) as tc:
        with tc.tile_pool(name="p", bufs=1) as pool:
            t = pool.tile([128, 128], mybir.dt.float32)
            nc.sync.dma_start(t[:], a_d[:])
            nc.sync.dma_start(o_d[:], t[:])
    nc.compile()
    return nc


def measure_exec_time_ns(inputs, iters=16):
    """Best-effort HW kernel time: single-core steady-state wall-clock of the
    jitted body minus a trivial kernel measured identically (axon RPC base is
    ~70 ms; device execution partially pipelines under it, so this is a lower
    bound; the TimelineSim cost-model estimate is printed alongside)."""
    import time
    import numpy as np

    nc = _get_nc()
    in_maps = _shard_inputs(
        np.asarray(inputs["x0"], np.float32),
        np.asarray(inputs["x1"], np.float32),
        np.asarray(inputs["Wqk"], np.float32),
        np.asarray(inputs["Wv"], np.float32),
        np.asarray(inputs["Wmerge"], np.float32),
    )
    run_full = _make_runner(nc, in_maps[:1])
    nc2 = _trivial_nc()
    run_tiny = _make_runner(nc2, [{"tin": np.zeros((128, 128), np.float32)}])
    run_full()
    run_tiny()

    # interleave full/trivial so axon RPC base drift cancels pairwise
    diffs = []
    fulls, tinys = [], []
    for _ in range(iters):
        t0 = time.perf_counter()
        run_full()
        t1 = time.perf_counter()
        run_tiny()
        t2 = time.perf_counter()
        fulls.append(t1 - t0)
        tinys.append(t2 - t1)
        diffs.append((t1 - t0) - (t2 - t1))
    diffs.sort()
    d = diffs[len(diffs) // 2]
    print(
        f"steady-state 1-core: full={sorted(fulls)[len(fulls)//2]*1e3:.2f} ms, "
        f"trivial={sorted(tinys)[len(tinys)//2]*1e3:.2f} ms, paired diff={d*1e6:.0f} us"
    )
    est = None
    try:
        from concourse.timeline_sim import TimelineSim

        est = TimelineSim(nc).simulate()
        print(f"TimelineSim estimate: {est:.0f} ns/core")
    except Exception:
        pass
    if d <= 1e-5 and est is not None:
        # measurement swamped by RPC noise; report the cost-model estimate
        return int(est)
    return int(d * 1e9)



# revision 38
# speedup vs baseline: 1.5890x; 1.5890x over previous
"""CrossBidirectionalAttention Trainium2 kernel (v2).

Problem (reference.py):
  B=2, L=S=2048, C=1024, H=16, HD=64
  qk0 = split_heads(x0 @ Wqk.T) * temp^0.5 ; qk1 likewise
  sim = einsum('blhd,bshd->bhls', qk0, qk1)
  o0 = softmax(sim, -1) @ v1 ; o1 = softmax(sim, -2)^T-contract @ v0
  return (merge(o0) @ Wmerge.T, merge(o1) @ Wmerge.T)

Sharding: 8 cores = 2 batches x 4 head-groups (4 heads each). Each core
computes its (b, head-group) slice end-to-end, producing partial merged
outputs (sum over its heads' columns of Wmerge); host sums the 4 partials
per batch. No max-subtraction is needed in softmax: sim ~ N(0,1) so
exp(temp*sim) <= ~e^6, safely in bf16 range.

v2 changes vs v1 (331.6us):
 - heads mapped to PE row-halves for sim (tile_position=(64*(h%2), 0)), so
   qk needs no partition-duplication -> no scalar/vector dup copies.
 - ScalarE (ACT) runs ONLY exp; all PSUM->SBUF staging copies are on DVE
   (plus ACT at the merge tail where it is otherwise idle).
 - softmax-denominator reciprocal rows are partition-broadcast on the idle
   GpSimd engine instead of PE broadcast-matmuls (frees PE + PSUM banks).
 - the o0 (E^T) accumulation is interleaved into the per-head lt loop in
   512-column chunks gated on the per-l-tile E^T DMA transposes, removing
   the per-head phase barrier; PSUM plan: po1 [128,2048] (4 banks) +
   3x sim [128,512] (3 banks) + po0 [128,512] (1 bank) = 8 banks exactly.
 - normalization sums still come free as a 65th ones-column in the
   attention matmuls' lhsT (cols beyond 65 are unused; matmul cost is
   N-bound so the narrow M is irrelevant).
"""

import os
import numpy as np
import ml_dtypes

B, L, S, C, H = 2, 2048, 2048, 1024, 16
HD = C // H  # 64
TEMP = float(HD) ** -0.5
N_CORES = 8
BF16 = ml_dtypes.bfloat16

_NC_CACHE = {}


def _build(lt_count=16, st_count=16):
    """Build the (identical-on-every-core) Bass program.

    lt_count/st_count: number of 128-row tiles of L and S (16 for the real
    problem; 4 for small validation runs). Must be multiples of 4.
    """
    import concourse.tile as tile
    from concourse import bacc, mybir

    assert lt_count % 4 == 0 and st_count % 4 == 0

    Lc = lt_count * 128
    Sc = st_count * 128
    n_sch = Sc // 512  # 512-col s-chunks per l-tile row
    n_q = lt_count // 4  # 512-col l-chunks for the o0 accumulation

    f32 = mybir.dt.float32
    bf16 = mybir.dt.bfloat16
    EXP = mybir.ActivationFunctionType.Exp
    MULT = mybir.AluOpType.mult

    nc = bacc.Bacc(None, target_bir_lowering=False, enable_partition_id=False)

    x0t_d = nc.dram_tensor("x0t", [128, 8, Lc], bf16, kind="ExternalInput")
    x1t_d = nc.dram_tensor("x1t", [128, 8, Sc], bf16, kind="ExternalInput")
    wqk_d = nc.dram_tensor("wqk", [128, 8, 256], bf16, kind="ExternalInput")
    wv_d = nc.dram_tensor("wv", [128, 8, 256], bf16, kind="ExternalInput")
    wm_d = nc.dram_tensor("wm", [128, 2, 1024], bf16, kind="ExternalInput")
    out0_d = nc.dram_tensor("out0", [128, lt_count, 1024], bf16, kind="ExternalOutput")
    out1_d = nc.dram_tensor("out1", [128, st_count, 1024], bf16, kind="ExternalOutput")

    with tile.TileContext(nc) as tc:
        with (
            tc.tile_pool(name="w", bufs=1) as wpool,
            tc.tile_pool(name="qk", bufs=1) as qkpool,
            tc.tile_pool(name="v", bufs=1) as vpool,
            tc.tile_pool(name="ont", bufs=1) as opool,
            tc.tile_pool(name="E", bufs=4) as epool,
            tc.tile_pool(name="et", bufs=1) as etpool,
            tc.tile_pool(name="rc", bufs=2) as rcpool,
            tc.tile_pool(name="rcb", bufs=2) as rcbpool,
            tc.tile_pool(name="ostage", bufs=4) as ostagepool,
            tc.tile_pool(name="psbig", bufs=1, space="PSUM") as psbig,
            tc.tile_pool(name="pss", bufs=3, space="PSUM") as pss,
            tc.tile_pool(name="psm", bufs=1, space="PSUM") as psm,
        ):
            wqk_t = wpool.tile([128, 8, 256], bf16)
            wv_t = wpool.tile([128, 8, 256], bf16)
            wm_t = wpool.tile([128, 2, 1024], bf16)
            # wqk m=0 columns lead the serial DMA chain so the first
            # projection matmul can start as early as possible; the other
            # weights are interleaved between the x quarters they gate.
            nc.sync.dma_start(wqk_t[:, :, 0:128], wqk_d[:, :, 0:128])

            qk0 = qkpool.tile([128, 2, Lc], bf16)
            qk1 = qkpool.tile([128, 2, Sc], bf16)
            v0 = vpool.tile([128, lt_count, 4, 65], bf16)
            v1 = vpool.tile([128, st_count, 4, 65], bf16)
            o0nT = opool.tile([128, 2, Lc], bf16)
            o1nT = opool.tile([128, 2, Sc], bf16)

            nc.vector.memset(v0[:, :, :, 64:65], 1.0)
            nc.vector.memset(v1[:, :, :, 64:65], 1.0)

            # x0T/x1T live in four 512-column-group quarter slots, each later
            # reused for the matching quarter of E^T (po0 chunk q reads only
            # l-cols [512q, 512q+512) = quarter q). Per-quarter slot sharing
            # lets a quarter's transposes start as soon as that quarter's x
            # readers are done instead of waiting for all projections.
            assert Lc == Sc
            n_g = Lc // 512
            xq = [
                etpool.tile([128, 16, 512], bf16, tag=f"et{g}", name=f"xq_{g}")
                for g in range(n_g)
            ]
            for g in range(n_g):
                nc.sync.dma_start(
                    xq[g][:, 0:8, :], x1t_d[:, :, g * 512 : g * 512 + 512]
                )
                nc.sync.dma_start(
                    xq[g][:, 8:16, :], x0t_d[:, :, g * 512 : g * 512 + 512]
                )
                if g == 0:
                    nc.sync.dma_start(wv_t[:], wv_d[:])
                elif g == 1:
                    nc.sync.dma_start(wqk_t[:, :, 128:256], wqk_d[:, :, 128:256])
                elif g == 2:
                    nc.sync.dma_start(wm_t[:], wm_d[:])
            if n_g < 2:
                nc.sync.dma_start(wqk_t[:, :, 128:256], wqk_d[:, :, 128:256])
            if n_g < 3:
                nc.sync.dma_start(wm_t[:], wm_d[:])

            # ---------------- projections ----------------
            # stream 0 = x0 (rows 8:16 of each quarter), stream 1 = x1 (0:8)
            def xrow(stream, g, k):
                return xq[g][:, (8 if stream == 0 else 0) + k, :]

            def proj_qk_chunk(stream, qk_t, m, cpos):
                # qkT: [head-col, seq] for heads (2m, 2m+1), one 512-col chunk
                g = cpos // 512
                ps = pss.tile(
                    [128, 512], f32, tag="ps", name=f"pqk_{qk_t.name}_{m}_{cpos}"
                )
                for k in range(8):
                    nc.tensor.matmul(
                        ps[:],
                        wqk_t[:, k, m * 128 : (m + 1) * 128],
                        xrow(stream, g, k),
                        start=(k == 0),
                        stop=(k == 7),
                    )
                nc.vector.tensor_copy(qk_t[:, m, cpos : cpos + 512], ps[:])

            def proj_v_mt(stream, v_t, mt):
                # v: [seq, head*65] ; one 128-seq-row tile
                g, col = mt // 4, (mt % 4) * 128
                ps = pss.tile([128, 512], f32, tag="ps", name=f"pv_{v_t.name}_{mt}")
                for k in range(8):
                    nc.tensor.matmul(
                        ps[:, 0:256],
                        xrow(stream, g, k)[:, col : col + 128],
                        wv_t[:, k, :],
                        start=(k == 0),
                        stop=(k == 7),
                    )
                nc.vector.tensor_copy(
                    v_t[:, mt, :, 0:64],
                    ps[:, 0:256].rearrange("p (h d) -> p h d", h=4),
                )

            def proj_qk(stream, n_seq, qk_t, m):
                for cpos in range(0, n_seq * 128, 512):
                    proj_qk_chunk(stream, qk_t, m, cpos)

            def proj_v(stream, n_seq, v_t):
                for mt in range(n_seq):
                    proj_v_mt(stream, v_t, mt)

            # Phase A: the projections head 0 needs up front, emitted in
            # x-quarter order to pipeline with the serial x-load DMA chain.
            # The rest (v1, qk m=1) are emitted as PE filler interleaved
            # into head 0's ACT-paced lt loop.
            for g in range(n_g):
                proj_qk_chunk(1, qk1, 0, g * 512)
                proj_qk_chunk(0, qk0, 0, g * 512)
                for mt in range(4 * g, min(4 * g + 4, lt_count)):
                    proj_v_mt(0, v0, mt)

            # Fillers ordered by ascending x column group: head 0's E^T
            # transposes overwrite the x quarters group by group, so the
            # readers of low column groups must drain first.
            fillers = []
            for g in range(n_g):
                for mt in range(4 * g, min(4 * g + 4, st_count)):
                    fillers.append(lambda mt=mt: proj_v_mt(1, v1, mt))
                fillers.append(lambda cpos=g * 512: proj_qk_chunk(1, qk1, 1, cpos))
                fillers.append(lambda cpos=g * 512: proj_qk_chunk(0, qk0, 1, cpos))

            # ---------------- per-head attention ----------------
            # E^T quarters: etq[g][sp, st, l'] = E[512g+l', st*128+sp],
            # allocated lazily into the x quarter slots.
            etq = {}

            def get_etq(g):
                if g not in etq:
                    etq[g] = etpool.tile(
                        [128, 16, 512], bf16, tag=f"et{g}", name=f"etq_{g}"
                    )
                return etq[g]

            def norm_chunk(po, dst, cpos, clen, nm):
                """dst[:, cpos:cpos+clen] = po[0:64] * 1/po[64] (row bcast).

                Done in 512-col pieces so region-level deps release the po
                PSUM banks incrementally (the next head's accumulation into
                the same slot can start as soon as its region is free).
                """
                for i, p0 in enumerate(range(0, clen, 512)):
                    pl = min(512, clen - p0)
                    rc = rcpool.tile([1, 512], f32, tag="rc", name=f"rc_{nm}_{i}")
                    rcb = rcbpool.tile([64, 512], f32, tag="rcb", name=f"rcb_{nm}_{i}")
                    nc.vector.reciprocal(rc[:, 0:pl], po[64:65, p0 : p0 + pl])
                    nc.gpsimd.partition_broadcast(rcb[:, 0:pl], rc[:, 0:pl])
                    nc.vector.tensor_tensor(
                        dst[:, cpos + p0 : cpos + p0 + pl],
                        po[0:64, p0 : p0 + pl],
                        rcb[:, 0:pl],
                        MULT,
                    )

            # The PE queue is (nearly) in-order, so every PE instruction is
            # emitted only once its inputs are (pipeline-)guaranteed ready:
            #  - po1 accumulation runs one l-tile behind the sim/exp stream
            #  - po0 work (gated on the E^T DMA transposes) is split into
            #    4-matmul subgroups, emitted at a smooth 1-2 per l-tile from
            #    a global queue that spills across head boundaries (head 0's
            #    chunks all spill into head 1 because its transposes wait for
            #    the x slot to be fully consumed by the filler projections).
            po1_t = {}
            e_ts = {}
            po0_t = {}

            def emit_po1(h, lt):
                hp = (h % 2) * 64
                e_t = e_ts.pop((h, lt))
                # at lt==0 the psbig slot regions are freed in c0..c3 norm
                # order by the previous head; starting with c1 gives the
                # chain a head start without stalling on c0
                cps = list(range(0, Sc, 512))
                if lt == 0 and len(cps) > 1:
                    cps = cps[1:] + cps[:1]
                for cpos in cps:
                    nc.tensor.matmul(
                        po1_t[h][0:65, cpos : cpos + 512],
                        v0[:, lt, h, :],
                        e_t[:, cpos : cpos + 512],
                        start=(lt == 0),
                        stop=(lt == lt_count - 1),
                    )
                if lt == lt_count - 1:
                    hm = h // 2
                    norm_chunk(
                        po1_t[h], o1nT[hp : hp + 64, hm, :], 0, Sc, f"o1_{h}"
                    )

            def emit_po0_sub(h, q, g):
                """Subgroup g (st rows 4g..4g+3) of o0 chunk (h, q)."""
                hp = (h % 2) * 64
                hm = h // 2
                if g == 0:
                    po0_t[(h, q)] = psm.tile(
                        [128, 512], f32, tag="po0", name=f"po0_{h}_{q}"
                    )
                po0 = po0_t[(h, q)]
                for st in range(4 * g, min(4 * g + 4, st_count)):
                    nc.tensor.matmul(
                        po0[0:65, :],
                        v1[:, st, h, :],
                        get_etq(q)[:, st, :],
                        start=(st == 0),
                        stop=(st == st_count - 1),
                    )
                if 4 * g + 4 >= st_count:
                    norm_chunk(
                        po0_t.pop((h, q)),
                        o0nT[hp : hp + 64, hm, :],
                        q * 512,
                        512,
                        f"o0_{h}_{q}",
                    )

            po0_piece = [None]

            def emit_po0_piece(plt):
                """o0 accumulation+norm for head 3, l-cols [128*plt, +128).

                Covers the last quarter in per-l-tile pieces, each gated on
                a single E^T transpose, so the end-of-kernel chain is one
                128-col piece instead of a full 512-col chunk.
                """
                h, q = 3, n_q - 1
                hp = (h % 2) * 64
                hm = h // 2
                j = plt % 4
                if po0_piece[0] is None:
                    po0_piece[0] = psm.tile(
                        [128, 512], f32, tag="po0", name="po0_piece"
                    )
                po0 = po0_piece[0]
                for st in range(st_count):
                    nc.tensor.matmul(
                        po0[0:65, j * 128 : (j + 1) * 128],
                        v1[:, st, h, :],
                        get_etq(q)[:, st, j * 128 : (j + 1) * 128],
                        start=(st == 0),
                        stop=(st == st_count - 1),
                    )
                norm_chunk(
                    po0[:, j * 128 : (j + 1) * 128],
                    o0nT[hp : hp + 64, hm, :],
                    plt * 128,
                    128,
                    f"o0p_{plt}",
                )

            n_sub = (st_count + 3) // 4
            subq = []  # (h, q, g) units in emission order
            glt = 0
            for h in range(4):
                hp = (h % 2) * 64
                hm = h // 2
                po1_t[h] = psbig.tile([128, Sc], f32, tag="po1", name=f"po1_{h}")
                for lt in range(lt_count):
                    e_t = epool.tile([128, Sc], bf16, tag="E", name=f"e_{h}_{lt}")
                    e_ts[(h, lt)] = e_t

                    def sim_chunk(cpos, e_t=e_t, lt=lt, hp=hp, hm=hm, h=h):
                        ps = pss.tile(
                            [128, 512], f32, tag="ps", name=f"sim_{h}_{lt}_{cpos}"
                        )
                        nc.tensor.matmul(
                            ps[:],
                            qk0[hp : hp + 64, hm, lt * 128 : (lt + 1) * 128],
                            qk1[hp : hp + 64, hm, cpos : cpos + 512],
                            start=True,
                            stop=True,
                            tile_position=(hp, 0),
                        )
                        nc.scalar.activation(
                            e_t[:, cpos : cpos + 512], ps[:], EXP, scale=TEMP
                        )

                    # sim c3 reuses the PSUM slot of sim c0 (3 rotating
                    # slots), so the lagged po1 block and fillers are
                    # emitted between them to cover exp(c0)'s latency.
                    for cpos in range(0, Sc - 512, 512):
                        sim_chunk(cpos)
                    if lt > 0:
                        emit_po1(h, lt - 1)
                    elif h > 0:
                        emit_po1(h - 1, lt_count - 1)
                    for _ in range(2):
                        if fillers:
                            fillers.pop(0)()
                    sim_chunk(Sc - 512)
                    # CORRECTNESS: a previous head's po0 reads of quarter
                    # lt//4 must be emitted before this head's transpose
                    # overwrites it (same-tile deps follow program order).
                    while subq and subq[0][0] < h and subq[0][1] <= lt // 4:
                        emit_po0_sub(*subq.pop(0))
                    col = (lt % 4) * 128
                    nc.sync.dma_start_transpose(
                        get_etq(lt // 4)[:, 0:st_count, col : col + 128], e_t[:]
                    )
                    # head 0's transposes wait on the x slot (consumed by
                    # fillers), so its po0 work spills into head 1. The
                    # last head's last quarter is handled in per-l-tile
                    # pieces below to shorten the end-of-kernel chain.
                    if lt % 4 == 3 and h > 0 and not (h == 3 and lt == lt_count - 1):
                        for g in range(n_sub):
                            subq.append((h, lt // 4, g))
                    n_emit = 0 if h == 0 else (2 if len(subq) > n_sub + 1 else 1)
                    for _ in range(n_emit):
                        if subq:
                            emit_po0_sub(*subq.pop(0))
                    if h == 3 and lt >= lt_count - 2:
                        # all chunk work must precede the pieces: the piece
                        # psm-slot rotation waits on the last chunk's banks
                        while subq:
                            emit_po0_sub(*subq.pop(0))
                        emit_po0_piece(lt - 2)
                    glt += 1
                if h == 0:
                    # queue head 0's po0 work now that the fillers are done
                    for q in range(n_q):
                        for g in range(n_sub):
                            subq.append((0, q, g))
            emit_po1(3, lt_count - 1)
            for unit in subq:
                emit_po0_sub(*unit)
            emit_po0_piece(lt_count - 2)
            emit_po0_piece(lt_count - 1)

            # ---------------- merge ----------------
            # Emission ordered by operand readiness: o0nT columns for chunks
            # q0..q2 of the last head normalize during its lt loop, o1nT only
            # at the flush, and o0nT's final 512 columns (q3, gated on the
            # last E^T transposes) come last. Ordering this way keeps PE fed
            # and starts the output DMAs as early as possible.
            nco = 0
            tail0 = lt_count - 4 if lt_count >= 8 else lt_count
            merge_groups = []
            for mt0 in range(0, tail0, 2):
                merge_groups.append((o0nT, out0_d, lt_count, mt0, 2))
            for mt0 in range(0, st_count, 2):
                merge_groups.append((o1nT, out1_d, st_count, mt0, 2))
            for mt0 in range(tail0, lt_count):
                merge_groups.append((o0nT, out0_d, lt_count, mt0, 1))
            for src, dst, n_seq, mt0, sz in merge_groups:
                if True:
                    mts = [mt for mt in range(mt0, mt0 + sz) if mt < n_seq]
                    st_t = ostagepool.tile(
                        [128, 2, 1024], bf16, tag="ostage", name=f"ost_{dst.name}_{mt0}"
                    )
                    for j, mt in enumerate(mts):
                        for c2 in range(2):
                            ps = pss.tile(
                                [128, 512], f32, tag="ps", name=f"mg_{dst.name}_{mt}_{c2}"
                            )
                            for k in range(2):
                                nc.tensor.matmul(
                                    ps[:],
                                    src[:, k, mt * 128 : (mt + 1) * 128],
                                    wm_t[:, k, c2 * 512 : (c2 + 1) * 512],
                                    start=(k == 0),
                                    stop=(k == 1),
                                )
                            if nco % 3 == 0:
                                nc.vector.tensor_copy(
                                    st_t[:, j, c2 * 512 : (c2 + 1) * 512], ps[:]
                                )
                            else:
                                nc.scalar.copy(
                                    st_t[:, j, c2 * 512 : (c2 + 1) * 512], ps[:]
                                )
                            nco += 1
                    nc.sync.dma_start(
                        dst[:, mt0 : mt0 + len(mts), :], st_t[:, 0 : len(mts), :]
                    )

    nc.compile()
    return nc


def _get_nc(lt_count=16, st_count=16):
    key = (lt_count, st_count)
    if key not in _NC_CACHE:
        _NC_CACHE[key] = _build(lt_count, st_count)
    return _NC_CACHE[key]


def _shard_inputs(x0, x1, Wqk, Wv, Wmerge, lt_count=16, st_count=16):
    """Host-side prep: per-core transposed bf16 shards."""
    Lc, Sc = lt_count * 128, st_count * 128
    in_maps = []
    for c in range(N_CORES):
        b = c // 4
        hg = c % 4
        hs = slice(hg * 256, (hg + 1) * 256)
        x0t = x0[b].T.reshape(8, 128, Lc).transpose(1, 0, 2)
        x1t = x1[b].T.reshape(8, 128, Sc).transpose(1, 0, 2)
        wqk = Wqk[hs, :].T.reshape(8, 128, 256).transpose(1, 0, 2)
        wv = Wv[hs, :].T.reshape(8, 128, 256).transpose(1, 0, 2)
        wm = Wmerge[:, hs].T.reshape(2, 128, 1024).transpose(1, 0, 2)
        in_maps.append(
            {
                "x0t": np.ascontiguousarray(x0t).astype(BF16),
                "x1t": np.ascontiguousarray(x1t).astype(BF16),
                "wqk": np.ascontiguousarray(wqk).astype(BF16),
                "wv": np.ascontiguousarray(wv).astype(BF16),
                "wm": np.ascontiguousarray(wm).astype(BF16),
            }
        )
    return in_maps


def _gather_outputs(results, lt_count=16, st_count=16):
    Lc, Sc = lt_count * 128, st_count * 128
    o0 = np.zeros((B, Lc, C), np.float32)
    o1 = np.zeros((B, Sc, C), np.float32)
    for c, res in enumerate(results):
        b = c // 4
        o0[b] += res["out0"].astype(np.float32).transpose(1, 0, 2).reshape(Lc, C)
        o1[b] += res["out1"].astype(np.float32).transpose(1, 0, 2).reshape(Sc, C)
    return o0, o1


def kernel(x0, x1, Wqk, Wv, Wmerge):
    from concourse.bass_utils import run_bass_kernel_spmd

    x0 = np.asarray(x0, dtype=np.float32)
    x1 = np.asarray(x1, dtype=np.float32)
    Wqk = np.asarray(Wqk, dtype=np.float32)
    Wv = np.asarray(Wv, dtype=np.float32)
    Wmerge = np.asarray(Wmerge, dtype=np.float32)

    nc = _get_nc()
    in_maps = _shard_inputs(x0, x1, Wqk, Wv, Wmerge)
    trace = os.environ.get("BENCH_TRACE", "") == "1"
    res = run_bass_kernel_spmd(
        nc, in_maps, core_ids=list(range(N_CORES)), trace=trace
    )
    if trace and res.exec_time_ns is not None:
        print(f"HW exec time: {res.exec_time_ns} ns")
        if res.instructions_and_trace is not None:
            print(f"trace: {res.instructions_and_trace[1]}")
    return _gather_outputs(res.results)


# ---------------------------------------------------------------------------
# Timing harness (test.py only): repeated steady-state executions of the
# jitted SPMD body with device-resident inputs, calibrated against a trivial
# kernel measured the same way to subtract axon dispatch/RPC overhead.

def _make_runner(nc, in_maps):
    import jax
    import numpy as np
    from jax.sharding import Mesh, PartitionSpec
    from jax.experimental.shard_map import shard_map
    from concourse import bass2jax, mybir

    bass2jax.install_neuronx_cc_hook()

    in_names, out_names, out_avals, zero_outs = [], [], [], []
    for alloc in nc.m.functions[0].allocations:
        if not isinstance(alloc, mybir.MemoryLocationSet):
            continue
        name = alloc.memorylocations[0].name
        if alloc.kind == "ExternalInput":
            in_names.append(name)
        elif alloc.kind == "ExternalOutput":
            out_names.append(name)
            dt = mybir.dt.np(alloc.dtype)
            out_avals.append(
                jax.core.ShapedArray(tuple(alloc.tensor_shape), dt)
            )
            zero_outs.append(np.zeros(tuple(alloc.tensor_shape), dt))
    n_params = len(in_names)
    all_names = in_names + out_names

    def _body(*args):
        outs = bass2jax._bass_exec_p.bind(
            *args,
            out_avals=tuple(out_avals),
            in_names=tuple(all_names),
            out_names=tuple(out_names),
            lowering_input_output_aliases=(),
            sim_require_finite=True,
            sim_require_nnan=True,
            nc=nc,
        )
        return tuple(outs)

    n_cores = len(in_maps)
    devices = jax.devices()[:n_cores]
    mesh = Mesh(np.asarray(devices), ("core",))
    n_out = len(out_names)
    sharded = jax.jit(
        shard_map(
            _body,
            mesh=mesh,
            in_specs=(PartitionSpec("core"),) * (n_params + n_out),
            out_specs=(PartitionSpec("core"),) * n_out,
            check_rep=False,
        ),
        keep_unused=True,
    )
    concat_in = [
        np.concatenate([np.asarray(in_maps[c][nm]) for c in range(n_cores)], axis=0)
        for nm in in_names
    ]
    concat_zero = [
        np.zeros((n_cores * z.shape[0], *z.shape[1:]), z.dtype) for z in zero_outs
    ]
    dev_args = [jax.device_put(a) for a in concat_in + concat_zero]

    def run():
        outs = sharded(*dev_args)
        jax.block_until_ready(outs)
        return outs

    return run


def _trivial_nc():
    import concourse.tile as tile
    from concourse import bacc, mybir

    nc = bacc.Bacc(None, target_bir_lowering=False, enable_partition_id=False)
    a_d = nc.dram_tensor("tin", [128, 128], mybir.dt.float32, kind="ExternalInput")
    o_d = nc.dram_tensor("tout", [128, 128], mybir.dt.float32, kind="ExternalOutput")
    with tile.TileContext(nc) as tc:
        with tc.tile_pool(name="p", bufs=1) as pool:
            t = pool.tile([128, 128], mybir.dt.float32)
            nc.sync.dma_start(t[:], a_d[:])
            nc.sync.dma_start(o_d[:], t[:])
    nc.compile()
    return nc


def measure_exec_time_ns(inputs, iters=16):
    """Best-effort HW kernel time: single-core steady-state wall-clock of the
    jitted body minus a trivial kernel measured identically (axon RPC base is
    ~70 ms; device execution partially pipelines under it, so this is a lower
    bound; the TimelineSim cost-model estimate is printed alongside)."""
    import time
    import numpy as np

    nc = _get_nc()
    in_maps = _shard_inputs(
        np.asarray(inputs["x0"], np.float32),
        np.asarray(inputs["x1"], np.float32),
        np.asarray(inputs["Wqk"], np.float32),
        np.asarray(inputs["Wv"], np.float32),
        np.asarray(inputs["Wmerge"], np.float32),
    )
    run_full = _make_runner(nc, in_maps[:1])
    nc2 = _trivial_nc()
    run_tiny = _make_runner(nc2, [{"tin": np.zeros((128, 128), np.float32)}])
    run_full()
    run_tiny()

    # interleave full/trivial so axon RPC base drift cancels pairwise
    diffs = []
    fulls, tinys = [], []
    for _ in range(iters):
        t0 = time.perf_counter()
        run_full()
        t1 = time.perf_counter()
        run_tiny()
        t2 = time.perf_counter()
        fulls.append(t1 - t0)
        tinys.append(t2 - t1)
        diffs.append((t1 - t0) - (t2 - t1))
    diffs.sort()
    d = diffs[len(diffs) // 2]
    print(
        f"steady-state 1-core: full={sorted(fulls)[len(fulls)//2]*1e3:.2f} ms, "
        f"trivial={sorted(tinys)[len(tinys)//2]*1e3:.2f} ms, paired diff={d*1e6:.0f} us"
    )
    est = None
    try:
        from concourse.timeline_sim import TimelineSim

        est = TimelineSim(nc).simulate()
        print(f"TimelineSim estimate: {est:.0f} ns/core")
    except Exception:
        pass
    if d <= 1e-5 and est is not None:
        # measurement swamped by RPC noise; report the cost-model estimate
        return int(est)
    return int(d * 1e9)
